# revision 1
# baseline (speedup 1.0000x reference)
# Multi-head causal attention (B=4, S=2048, D=1024, H=16) on 8 TRN2 NeuronCores.
#
# Sharding: batch x query-chunk. Core c handles batch b=c//2 and two 512-row
# query chunks of that batch: cores with c%2==0 take real chunks (0, 3),
# c%2==1 take (1, 2). The SPMD program is identical on every core: it
# processes two query "slots" with fixed kk-tile capacities (8, 16); real
# chunk needs (4,8,12,16 tiles) are mapped into those capacities and the
# excess key tiles are zeroed by per-core causal-mask input data. Each core
# computes K/V projections for its whole batch (duplicated across the 2 cores
# sharing a batch) so no cross-core collectives are needed.
#
# Matmuls run in bf16 (fp32 PSUM accumulation); softmax statistics stay fp32.
# K/Q/V/OT all live in SBUF for the whole kernel -- no DRAM spills.
# Attention uses the transposed-scores layout St[kk, q]:
#   Kt[d, s], Qt[d, q]; St = Kt_tile.T @ Qt  (2 heads packed into one 2-bank
#   PSUM tile, exp'd in a single ACT op)
#   P = exp(St) * mask
#   OT[dv, q] += V_aug[kk, 65].T @ P   -- V carries a ones column, so PSUM
#     row 64 accumulates the softmax denominators for free.
#   OT_norm = OT * reciprocal(bcast(denoms)); y = sum_dc OT.T @ woT + b_o.
import sys

if '/opt/trn_rl_repo' not in sys.path:
    sys.path.insert(0, '/opt/trn_rl_repo')

import numpy as np

B, S, D = 4, 2048, 1024
H, DK = 16, 64
NCORES = 8
SC = 512
NKT = S // 128            # 16 kk tiles
HPN = D // 128            # 8 head-pairs
CAPS = (8, 16)            # kk-tile capacity per slot (uniform across cores)
CHUNKS = [(0, 3), (1, 2)]  # real chunk pair per core parity

_CACHE = {}


def _build_program():
    import contextlib

    import concourse.tile as tile
    from concourse import bacc, mybir

    F32 = mybir.dt.float32
    BF16 = mybir.dt.bfloat16
    EXP = mybir.ActivationFunctionType.Exp

    nc = bacc.Bacc("TRN2", target_bir_lowering=False, debug=False,
                   num_devices=NCORES)

    xT_d = nc.dram_tensor("xT", [D, S], BF16, kind="ExternalInput")
    xQT_d = nc.dram_tensor("xQT", [D, 2 * SC], BF16, kind="ExternalInput")
    wqT_d = nc.dram_tensor("wqT", [D, D], BF16, kind="ExternalInput")
    wkT_d = nc.dram_tensor("wkT", [D, D], BF16, kind="ExternalInput")
    wvT_d = nc.dram_tensor("wvT", [D, D], BF16, kind="ExternalInput")
    woT_d = nc.dram_tensor("woT", [D, D], BF16, kind="ExternalInput")
    bias_d = nc.dram_tensor("bias", [1, D], BF16, kind="ExternalInput")
    masks_d = nc.dram_tensor("masks", [128, NKT * 1024], BF16,
                             kind="ExternalInput")
    y_d = nc.dram_tensor("y", [2 * SC, D], F32, kind="ExternalOutput")

    with tile.TileContext(nc) as tc, contextlib.ExitStack() as ctx:
        smalls = ctx.enter_context(tc.tile_pool(name="smalls", bufs=1))
        p_OT = ctx.enter_context(tc.tile_pool(name="otp", bufs=1))
        p_Kt = ctx.enter_context(tc.tile_pool(name="ktp", bufs=1))
        p_Qt = ctx.enter_context(tc.tile_pool(name="qtp", bufs=1))
        p_V = ctx.enter_context(tc.tile_pool(name="vp", bufs=1))
        p_mk = ctx.enter_context(tc.tile_pool(name="mk", bufs=1))

        masks_sb = p_mk.tile([128, NKT * 1024], BF16, tag="masks")
        nc.gpsimd.dma_start(masks_sb[:], masks_d.ap())

        bias_sb = smalls.tile([1, D], BF16, tag="bias")
        nc.sync.dma_start(bias_sb[:], bias_d.ap())
        ones1f = smalls.tile([1, 128], F32, tag="ones1f")
        nc.vector.memset(ones1f[:], 1.0)
        ones1 = smalls.tile([1, 128], BF16, tag="ones1")
        nc.vector.tensor_copy(ones1[:], ones1f[:])
        ones256f = smalls.tile([128, 256], F32, tag="ones256f")
        nc.vector.memset(ones256f[:], 1.0)

        OT = p_OT.tile([128, HPN * 2 * SC], BF16, tag="OT")
        Kt = p_Kt.tile([128, HPN * S], BF16, tag="Kt")
        Qt = p_Qt.tile([128, HPN * 2 * SC], BF16, tag="Qt")
        Vsb = p_V.tile([128, NKT * H * 65], BF16, tag="Vsb")

        # ones columns of V_aug (all 16 s-tiles, one strided copy)
        nc.vector.tensor_copy(
            Vsb[:].rearrange("p (s h c) -> p s h c", s=NKT, c=65)
            [:, :, :, 64:65],
            ones256f[:].rearrange("p (s h) -> p s h", s=NKT)[:, :, :, None])

        # ---- V + K projections, one half of the sequence at a time ----
        with tc.tile_pool(name="xth", bufs=2) as p_xh, \
             tc.tile_pool(name="wfv", bufs=1) as p_wv, \
             tc.tile_pool(name="wfk", bufs=1) as p_wk, \
             tc.tile_pool(name="psp", bufs=8, space="PSUM") as psp:
            wv = p_wv.tile([128, 8 * D], BF16, tag="wv")
            wk = p_wk.tile([128, 8 * D], BF16, tag="wk")
            xhs = [p_xh.tile([128, 8 * 1024], BF16, tag="xh",
                             name=f"xh_{h}") for h in range(2)]
            # interleave so the first V matmul group's inputs land first
            for k in range(8):
                nc.sync.dma_start(
                    xhs[0][:, k * 1024:(k + 1) * 1024],
                    xT_d.ap()[k * 128:(k + 1) * 128, 0:1024])
                nc.sync.dma_start(
                    wv[:, k * D:(k + 1) * D],
                    wvT_d.ap()[k * 128:(k + 1) * 128, :])
            for k in range(8):
                nc.sync.dma_start(
                    wk[:, k * D:(k + 1) * D],
                    wkT_d.ap()[k * 128:(k + 1) * 128, :])
                nc.sync.dma_start(
                    xhs[1][:, k * 1024:(k + 1) * 1024],
                    xT_d.ap()[k * 128:(k + 1) * 128, 1024:2048])

            for half in range(2):
                xh = xhs[half]
                # V for the 8 s-tiles of this half (into SBUF V_aug layout)
                for sti in range(8):
                    st_g = half * 8 + sti
                    for dvc in range(2):
                        ps = psp.tile([128, 512], F32, tag="ps")
                        for k in range(8):
                            nc.tensor.matmul(
                                ps[:],
                                xh[:, k * 1024 + sti * 128:
                                   k * 1024 + (sti + 1) * 128],
                                wv[:, k * D + dvc * 512:k * D + (dvc + 1) * 512],
                                start=(k == 0), stop=(k == 7))
                        off = st_g * 1040 + dvc * 520
                        nc.vector.tensor_copy(
                            Vsb[:, off:off + 520]
                            .rearrange("p (h c) -> p h c", c=65)[:, :, 0:64],
                            ps[:].rearrange("p (h c) -> p h c", c=64))
                # K for the 2 s-chunks of this half -> SBUF-resident Kt
                for sc2 in range(2):
                    sc = half * 2 + sc2
                    ps8 = [psp.tile([128, 512], F32, tag="ps",
                                    name=f"psk_{sc}_{hp}")
                           for hp in range(HPN)]
                    for k in range(8):
                        for hp in range(HPN):
                            nc.tensor.matmul(
                                ps8[hp][:],
                                wk[:, k * D + hp * 128:k * D + (hp + 1) * 128],
                                xh[:, k * 1024 + sc2 * 512:
                                   k * 1024 + (sc2 + 1) * 512],
                                start=(k == 0), stop=(k == 7))
                    for hp in range(HPN):
                        nc.vector.tensor_copy(
                            Kt[:, hp * S + sc * 512:hp * S + (sc + 1) * 512],
                            ps8[hp][:])

        # ------------- Q projection (xQT streamed, wq resident) ----------
        with tc.tile_pool(name="wf2", bufs=1) as p_w2, \
             tc.tile_pool(name="xqs", bufs=4) as p_xq, \
             tc.tile_pool(name="psq", bufs=8, space="PSUM") as psq:
            wq = p_w2.tile([128, 8 * D], BF16, tag="w2")
            for k in range(8):
                nc.sync.dma_start(
                    wq[:, k * D:(k + 1) * D],
                    wqT_d.ap()[k * 128:(k + 1) * 128, :])
            for ci in range(2):
                ps8 = [psq.tile([128, 512], F32, tag="ps",
                                name=f"psq_{ci}_{hp}") for hp in range(HPN)]
                for k in range(8):
                    xq1 = p_xq.tile([128, 512], BF16, tag="xq")
                    nc.sync.dma_start(
                        xq1[:],
                        xQT_d.ap()[k * 128:(k + 1) * 128,
                                   ci * SC:(ci + 1) * SC])
                    for hp in range(HPN):
                        nc.tensor.matmul(
                            ps8[hp][:],
                            wq[:, k * D + hp * 128:k * D + (hp + 1) * 128],
                            xq1[:], start=(k == 0), stop=(k == 7))
                for hp in range(HPN):
                    nc.vector.tensor_copy(
                        Qt[:, hp * 2 * SC + ci * SC:
                           hp * 2 * SC + (ci + 1) * SC],
                        ps8[hp][:])

        # ------------- attention + interleaved output projection ---------
        with tc.tile_pool(name="rs", bufs=2) as p_rs, \
             tc.tile_pool(name="bcp", bufs=1) as p_bc, \
             tc.tile_pool(name="pp", bufs=8) as p_P, \
             tc.tile_pool(name="wo", bufs=1) as p_wo, \
             tc.tile_pool(name="ybp", bufs=4) as p_yb, \
             tc.tile_pool(name="pst", bufs=2, space="PSUM") as p_st, \
             tc.tile_pool(name="pav", bufs=4, space="PSUM") as p_av:

            wo = p_wo.tile([128, 8 * D], BF16, tag="wo")
            for k in range(8):
                nc.sync.dma_start(
                    wo[:, k * D:(k + 1) * D],
                    woT_d.ap()[k * 128:(k + 1) * 128, :])

            for ci, cap in enumerate(CAPS):
                for bl in range(HPN // 2):
                    av = [p_av.tile([128, 512], F32, tag="av",
                                    name=f"av_{ci}_{bl}_{i}")
                          for i in range(4)]

                    def emit_av(t, p_tiles, cap=cap, av=av, bl=bl):
                        for hp_i in range(2):
                            for hh in range(2):
                                hi = 2 * hp_i + hh
                                off = (t * 1040 + (2 * bl + hp_i) * 130 +
                                       hh * 65)
                                nc.tensor.matmul(
                                    av[hi][0:65, :],
                                    Vsb[:, off:off + 65],
                                    p_tiles[hp_i][:, hh * 512:(hh + 1) * 512],
                                    start=(t == 0), stop=(t == cap - 1))

                    pending = []
                    for t in range(cap):
                        p_cur = []
                        for hp_i in range(2):
                            hp = 2 * bl + hp_i
                            st = p_st.tile([128, 1024], F32, tag="st")
                            for hh in range(2):
                                r0 = 64 * hh
                                nc.tensor.matmul(
                                    st[:, hh * 512:(hh + 1) * 512],
                                    Kt[r0:r0 + 64,
                                       hp * S + t * 128:hp * S + (t + 1) * 128],
                                    Qt[r0:r0 + 64,
                                       hp * 2 * SC + ci * SC:
                                       hp * 2 * SC + (ci + 1) * SC],
                                    start=True, stop=True,
                                    tile_position=(r0, 0))
                            p1 = p_P.tile([128, 1024], BF16, tag="p")
                            nc.scalar.activation(p1[:], st[:], EXP)
                            if ci == 0 or t >= 8:
                                p2 = p_P.tile([128, 1024], BF16, tag="p")
                                nc.vector.tensor_mul(
                                    p2[:], p1[:],
                                    masks_sb[:, t * 1024:(t + 1) * 1024])
                                p1 = p2
                            p_cur.append(p1)
                        # lag-2 software pipeline: exp(t) overlaps the PE
                        # work of scores(t..t+1) + AV(t-2..t-1)
                        pending.append((t, p_cur))
                        if len(pending) > 2:
                            tt, pp_t = pending.pop(0)
                            emit_av(tt, pp_t)
                    for tt, pp_t in pending:
                        emit_av(tt, pp_t)
                    # normalize, one head-pair at a time
                    for hp_i in range(2):
                        hp = 2 * bl + hp_i
                        rs = p_rs.tile([1, 1024], F32, tag="rs")
                        for hh in range(2):
                            hi = 2 * hp_i + hh
                            nc.vector.tensor_copy(
                                rs[0:1, hh * 512:hh * 512 + 512],
                                av[hi][64:65, :])
                        bc = p_bc.tile([128, 1024], F32, tag="bc")
                        nc.gpsimd.partition_broadcast(bc[:], rs[:])
                        rbc = p_bc.tile([128, 1024], F32, tag="rbc")
                        scr = p_bc.tile([128, 1024], F32, tag="scr")
                        nc.vector.reciprocal_approx_accurate(
                            rbc[:], bc[:], scratch=scr[:])
                        for hh in range(2):
                            hi = 2 * hp_i + hh
                            r0 = 64 * hh
                            nc.vector.tensor_mul(
                                OT[r0:r0 + 64,
                                   hp * 2 * SC + ci * SC:
                                   hp * 2 * SC + (ci + 1) * SC],
                                av[hi][0:64, :],
                                rbc[r0:r0 + 64, hh * 512:hh * 512 + 512])

            # ---------------- output projection ----------------
            for qi in range(8):
                for nc2 in range(2):
                    ps = p_av.tile([128, 512], F32, tag="av",
                                   name=f"psy_{qi}_{nc2}")
                    for dc in range(8):
                        nc.tensor.matmul(
                            ps[:],
                            OT[:, dc * 2 * SC + qi * 128:
                               dc * 2 * SC + (qi + 1) * 128],
                            wo[:, dc * D + nc2 * 512:
                               dc * D + (nc2 + 1) * 512],
                            start=(dc == 0), stop=False)
                    nc.tensor.matmul(
                        ps[:], ones1[:],
                        bias_sb[0:1, nc2 * 512:(nc2 + 1) * 512],
                        start=False, stop=True)
                    yb = p_yb.tile([128, 512], F32, tag="yb")
                    nc.vector.tensor_copy(yb[:], ps[:])
                    nc.sync.dma_start(
                        y_d.ap()[qi * 128:(qi + 1) * 128,
                                 nc2 * 512:(nc2 + 1) * 512], yb[:])

    nc.compile()
    return nc


def _get_program():
    if 'nc' not in _CACHE:
        _CACHE['nc'] = _build_program()
    return _CACHE['nc']


def _tri_masks():
    p = np.arange(128)[:, None]
    f = np.arange(SC)[None, :]
    return [(p <= f - 128 * r).astype(np.float32) for r in range(4)]


def _masks_for_core(c):
    import ml_dtypes
    tri = _tri_masks()
    ones = np.ones((128, SC), np.float32)
    zeros = np.zeros((128, SC), np.float32)
    j_pair = CHUNKS[c % 2]
    out = np.zeros((128, NKT * 1024), np.float32)
    for ci, cap in enumerate(CAPS):
        j = j_pair[ci]
        t0 = 0 if ci == 0 else 8
        for t in range(t0, cap):
            if t < 4 * j:
                m = ones
            elif t < 4 * j + 4:
                m = tri[t - 4 * j]
            else:
                m = zeros
            out[:, t * 1024:t * 1024 + 512] = m
            out[:, t * 1024 + 512:(t + 1) * 1024] = m
    return out.astype(ml_dtypes.bfloat16)


def kernel(x, w_q, w_k, w_v, w_o, b_o):
    import ml_dtypes
    from concourse.bass_utils import run_bass_kernel_spmd

    BF = ml_dtypes.bfloat16
    x = np.asarray(x, dtype=np.float32)
    nc = _get_program()

    scale = np.float32(1.0 / np.sqrt(DK))
    common = {
        "wqT": np.ascontiguousarray(
            (np.asarray(w_q, np.float32).T * scale)).astype(BF),
        "wkT": np.ascontiguousarray(np.asarray(w_k, np.float32).T).astype(BF),
        "wvT": np.ascontiguousarray(np.asarray(w_v, np.float32).T).astype(BF),
        "woT": np.ascontiguousarray(np.asarray(w_o, np.float32).T).astype(BF),
        "bias": np.asarray(b_o, np.float32)[None, :].astype(BF),
    }

    in_maps = []
    for c in range(NCORES):
        b = c // 2
        j1, j2 = CHUNKS[c % 2]
        xb = x[b]
        xq = np.concatenate(
            [xb[j1 * SC:(j1 + 1) * SC], xb[j2 * SC:(j2 + 1) * SC]], axis=0)
        in_maps.append({
            "xT": np.ascontiguousarray(xb.T).astype(BF),
            "xQT": np.ascontiguousarray(xq.T).astype(BF),
            "masks": _masks_for_core(c),
            **common,
        })

    res = run_bass_kernel_spmd(nc, in_maps, core_ids=list(range(NCORES)),
                               trace=_CACHE.get('trace', False),
                               tmpdir=_CACHE.get('tmpdir'))
    _CACHE['last_res'] = res

    y = np.empty((B, S, D), dtype=np.float32)
    for c in range(NCORES):
        b = c // 2
        j1, j2 = CHUNKS[c % 2]
        yc = res.results[c]["y"]
        y[b, j1 * SC:(j1 + 1) * SC] = yc[0:SC]
        y[b, j2 * SC:(j2 + 1) * SC] = yc[SC:2 * SC]
    return y



# revision 2
# speedup vs baseline: 1.4011x; 1.4011x over previous
# Multi-head causal attention (B=4, S=2048, D=1024, H=16) on 8 TRN2 NeuronCores.
#
# Sharding: batch x head-half. Core c handles batch b=c//2 and heads
# [8p, 8p+8) where p=c%2 (d-model slice [512p, 512p+512)). Every core runs
# the identical causal program: Q/K/V projections for its 8 heads over the
# full sequence, causal attention for all 4 query chunks of 512, and a
# PARTIAL output projection y_part = ctx_local @ woT[512p:512p+512] + b_o/2.
# The host unshards by summing the two partial outputs of each batch pair.
# No cross-core collectives; zero duplicated projection work; causal
# structure is exploited exactly (diagonal tiles column-trimmed).
#
# Matmuls in bf16 (fp32 PSUM); softmax stats fp32. K/Q/V/OT SBUF-resident.
# Attention uses transposed scores St[kk, q]:
#   St = Kt_tile.T @ Qt (2 heads packed per PSUM tile via tile_position)
#   P = exp(St) (trimmed to the causal column window on diagonal tiles),
#   then P[:, 0:128(r+1)] *= [zeros|tri] mask (zeroes the stale/unwritten
#   region AND applies the diagonal triangle in one op)
#   OT[dv, q] += V_aug[kk, 65].T @ P  -- V carries a ones column so PSUM
#     row 64 accumulates softmax denominators for free.
#   OT_norm = OT * reciprocal(bcast(denoms)); per query-chunk the partial
#   y is produced right after normalization (projections for the NEXT
#   chunk are interleaved at block boundaries to keep PE fed while the
#   Scalar engine runs the exps).
import sys

if '/opt/trn_rl_repo' not in sys.path:
    sys.path.insert(0, '/opt/trn_rl_repo')

import numpy as np

B, S, D = 4, 2048, 1024
H, DK = 16, 64
NCORES = 8
SC = 512                 # query chunk
NHP = 4                  # local head-pairs per core (8 heads)
NCHUNK = S // SC         # 4 query chunks, chunk ci needs 4*(ci+1) kk tiles

_CACHE = {}


def _build_program():
    import contextlib

    import concourse.tile as tile
    from concourse import bacc, mybir

    F32 = mybir.dt.float32
    BF16 = mybir.dt.bfloat16
    EXP = mybir.ActivationFunctionType.Exp

    nc = bacc.Bacc("TRN2", target_bir_lowering=False, debug=False,
                   num_devices=NCORES)

    xT_d = nc.dram_tensor("xT", [D, S], BF16, kind="ExternalInput")
    wqT_d = nc.dram_tensor("wqT", [D, 512], BF16, kind="ExternalInput")
    wkT_d = nc.dram_tensor("wkT", [D, 512], BF16, kind="ExternalInput")
    wvT_d = nc.dram_tensor("wvT", [D, 512], BF16, kind="ExternalInput")
    woT_d = nc.dram_tensor("woT", [512, D], BF16, kind="ExternalInput")
    bias_d = nc.dram_tensor("bias", [1, D], BF16, kind="ExternalInput")
    masks_d = nc.dram_tensor("masks", [128, 4 * 1024], BF16,
                             kind="ExternalInput")
    y_d = nc.dram_tensor("y", [S, D], F32, kind="ExternalOutput")

    with tile.TileContext(nc) as tc, contextlib.ExitStack() as ctx:
        smalls = ctx.enter_context(tc.tile_pool(name="smalls", bufs=1))
        p_OT = ctx.enter_context(tc.tile_pool(name="otp", bufs=1))
        p_Kt = ctx.enter_context(tc.tile_pool(name="ktp", bufs=1))
        p_Qt = ctx.enter_context(tc.tile_pool(name="qtp", bufs=1))
        p_V = ctx.enter_context(tc.tile_pool(name="vp", bufs=1))
        p_mk = ctx.enter_context(tc.tile_pool(name="mk", bufs=1))
        p_w = ctx.enter_context(tc.tile_pool(name="wp", bufs=1))
        p_x = ctx.enter_context(tc.tile_pool(name="xp", bufs=2))
        p_rs = ctx.enter_context(tc.tile_pool(name="rs", bufs=2))
        p_bc = ctx.enter_context(tc.tile_pool(name="bcp", bufs=1))
        p_P = ctx.enter_context(tc.tile_pool(name="pp", bufs=8))
        p_yb = ctx.enter_context(tc.tile_pool(name="ybp", bufs=4))
        p_st = ctx.enter_context(tc.tile_pool(name="pst", bufs=2,
                                              space="PSUM"))
        p_av = ctx.enter_context(tc.tile_pool(name="pav", bufs=4,
                                              space="PSUM"))

        # ---------------- weights / constants DMA ----------------
        wk = p_w.tile([128, 8 * 512], BF16, tag="wk")
        wv = p_w.tile([128, 8 * 512], BF16, tag="wv")
        wq = p_w.tile([128, 8 * 512], BF16, tag="wq")
        for k in range(8):
            nc.sync.dma_start(wk[:, k * 512:(k + 1) * 512],
                              wkT_d.ap()[k * 128:(k + 1) * 128, :])
            nc.sync.dma_start(wv[:, k * 512:(k + 1) * 512],
                              wvT_d.ap()[k * 128:(k + 1) * 128, :])
        for k in range(8):
            nc.sync.dma_start(wq[:, k * 512:(k + 1) * 512],
                              wqT_d.ap()[k * 128:(k + 1) * 128, :])

        masks_sb = p_mk.tile([128, 4 * 1024], BF16, tag="masks")
        nc.gpsimd.dma_start(masks_sb[:], masks_d.ap())
        wo = p_w.tile([128, 4 * 1024], BF16, tag="wo")
        for k in range(4):
            nc.gpsimd.dma_start(wo[:, k * 1024:(k + 1) * 1024],
                                woT_d.ap()[k * 128:(k + 1) * 128, :])
        bias_sb = smalls.tile([1, D], BF16, tag="bias")
        nc.gpsimd.dma_start(bias_sb[:], bias_d.ap())
        biasbc = smalls.tile([128, D], BF16, tag="biasbc")
        nc.gpsimd.partition_broadcast(biasbc[:], bias_sb[:])

        onesf = smalls.tile([128, 128], F32, tag="onesf")
        nc.vector.memset(onesf[:], 1.0)

        OT = p_OT.tile([128, NHP * S], BF16, tag="OT")
        Kt = p_Kt.tile([128, NHP * S], BF16, tag="Kt")
        Qt = p_Qt.tile([128, NHP * S], BF16, tag="Qt")
        Vsb = p_V.tile([128, 16 * 8 * 65], BF16, tag="Vsb")

        # ones columns of V_aug (16 s-tiles x 8 heads, one strided copy)
        nc.vector.tensor_copy(
            Vsb[:].rearrange("p (s h c) -> p s h c", s=16, c=65)
            [:, :, :, 64:65],
            onesf[:].rearrange("p (s h) -> p s h", s=16)[:, :, :, None])

        # pre-zero the P pool ring so mask-muls never read NaN garbage
        pzero = []
        for i in range(8):
            pz = p_P.tile([128, 1024], BF16, tag="p", name=f"pz_{i}")
            nc.vector.memset(pz[:], 0.0)
            pzero.append(pz)

        # ---------------- projection stages ----------------
        # stage sc projects K/Q for s-chunk sc and V for s-tiles 4sc..4sc+3
        def emit_x_dma(sc):
            xch = p_x.tile([128, 8 * 512], BF16, tag="xch",
                           name=f"xch_{sc}")
            for k in range(8):
                nc.sync.dma_start(
                    xch[:, k * 512:(k + 1) * 512],
                    xT_d.ap()[k * 128:(k + 1) * 128,
                              sc * 512:(sc + 1) * 512])
            return xch

        def emit_proj_stage(sc, xch):
            # K: out [128 douts(hp), 512 s]
            for hp in range(NHP):
                ps = p_av.tile([128, 512], F32, tag="av",
                               name=f"psk_{sc}_{hp}")
                for k in range(8):
                    nc.tensor.matmul(
                        ps[:],
                        wk[:, k * 512 + hp * 128:k * 512 + (hp + 1) * 128],
                        xch[:, k * 512:(k + 1) * 512],
                        start=(k == 0), stop=(k == 7))
                nc.vector.tensor_copy(
                    Kt[:, hp * S + sc * 512:hp * S + (sc + 1) * 512], ps[:])
            # V: out [128 s, 512 douts] per s-tile
            for sti in range(4):
                st_g = 4 * sc + sti
                ps = p_av.tile([128, 512], F32, tag="av",
                               name=f"psv_{sc}_{sti}")
                for k in range(8):
                    nc.tensor.matmul(
                        ps[:],
                        xch[:, k * 512 + sti * 128:k * 512 + (sti + 1) * 128],
                        wv[:, k * 512:(k + 1) * 512],
                        start=(k == 0), stop=(k == 7))
                nc.vector.tensor_copy(
                    Vsb[:, st_g * 520:(st_g + 1) * 520]
                    .rearrange("p (h c) -> p h c", c=65)[:, :, 0:64],
                    ps[:].rearrange("p (h c) -> p h c", c=64))
            # Q: out [128 douts(hp), 512 q]
            for hp in range(NHP):
                ps = p_av.tile([128, 512], F32, tag="av",
                               name=f"psq_{sc}_{hp}")
                for k in range(8):
                    nc.tensor.matmul(
                        ps[:],
                        wq[:, k * 512 + hp * 128:k * 512 + (hp + 1) * 128],
                        xch[:, k * 512:(k + 1) * 512],
                        start=(k == 0), stop=(k == 7))
                nc.vector.tensor_copy(
                    Qt[:, hp * S + sc * 512:hp * S + (sc + 1) * 512], ps[:])

        xch0 = emit_x_dma(0)
        xch_next = emit_x_dma(1)
        emit_proj_stage(0, xch0)

        # ---------------- attention + interleaved proj/out-proj ----------
        for ci in range(NCHUNK):
            cap = 4 * (ci + 1)
            for bl in range(2):
                av = [p_av.tile([128, 512], F32, tag="av",
                                name=f"av_{ci}_{bl}_{i}")
                      for i in range(4)]

                def emit_av(t, p_tiles, cap=cap, av=av, bl=bl):
                    for hp_i in range(2):
                        for hh in range(2):
                            hi = 2 * hp_i + hh
                            h = (2 * bl + hp_i) * 2 + hh
                            off = t * 520 + h * 65
                            nc.tensor.matmul(
                                av[hi][0:65, :],
                                Vsb[:, off:off + 65],
                                p_tiles[hp_i][:, hh * 512:(hh + 1) * 512],
                                start=(t == 0), stop=(t == cap - 1))

                pending = []
                for t in range(cap):
                    r = t - 4 * ci  # >=0 -> diagonal tile
                    p_cur = []
                    for hp_i in range(2):
                        hp = 2 * bl + hp_i
                        st = p_st.tile([128, 1024], F32, tag="st")
                        q0 = 128 * r if r >= 0 else 0
                        for hh in range(2):
                            r0 = 64 * hh
                            nc.tensor.matmul(
                                st[:, hh * 512 + q0:(hh + 1) * 512],
                                Kt[r0:r0 + 64,
                                   hp * S + t * 128:hp * S + (t + 1) * 128],
                                Qt[r0:r0 + 64,
                                   hp * S + ci * SC + q0:
                                   hp * S + (ci + 1) * SC],
                                start=True, stop=True,
                                tile_position=(r0, 0))
                        p1 = p_P.tile([128, 1024], BF16, tag="p")
                        if r >= 1:
                            # trimmed exp: both hh windows in one strided ACT
                            w = 512 - q0
                            nc.scalar.activation(
                                p1[:].rearrange("p (h q) -> p h q", h=2)
                                [:, :, q0:512],
                                st[:].rearrange("p (h q) -> p h q", h=2)
                                [:, :, q0:512],
                                EXP)
                        else:
                            nc.scalar.activation(p1[:], st[:], EXP)
                        if r >= 0:
                            # zero [0,q0) (stale) + triangle on [q0,q0+128)
                            wm = q0 + 128
                            nc.vector.tensor_mul(
                                p1[:].rearrange("p (h q) -> p h q", h=2)
                                [:, :, 0:wm],
                                p1[:].rearrange("p (h q) -> p h q", h=2)
                                [:, :, 0:wm],
                                masks_sb[:, r * 1024:(r + 1) * 1024]
                                .rearrange("p (h q) -> p h q", h=2)
                                [:, :, 0:wm])
                        p_cur.append(p1)
                    # lag-2 software pipeline: exp(t) overlaps the PE
                    # work of scores(t..t+1) + AV(t-2..t-1)
                    pending.append((t, p_cur))
                    if len(pending) > 2:
                        tt, pp_t = pending.pop(0)
                        emit_av(tt, pp_t)
                for tt, pp_t in pending:
                    emit_av(tt, pp_t)

                # normalize, one head-pair at a time
                for hp_i in range(2):
                    hp = 2 * bl + hp_i
                    rs = p_rs.tile([1, 1024], F32, tag="rs")
                    for hh in range(2):
                        hi = 2 * hp_i + hh
                        nc.vector.tensor_copy(
                            rs[0:1, hh * 512:hh * 512 + 512],
                            av[hi][64:65, :])
                    bc = p_bc.tile([128, 1024], F32, tag="bc")
                    nc.gpsimd.partition_broadcast(bc[:], rs[:])
                    rbc = p_bc.tile([128, 1024], F32, tag="rbc")
                    scr = p_bc.tile([128, 1024], F32, tag="scr")
                    nc.vector.reciprocal_approx_accurate(
                        rbc[:], bc[:], scratch=scr[:])
                    for hh in range(2):
                        hi = 2 * hp_i + hh
                        r0 = 64 * hh
                        nc.vector.tensor_mul(
                            OT[r0:r0 + 64,
                               hp * S + ci * SC:hp * S + (ci + 1) * SC],
                            av[hi][0:64, :],
                            rbc[r0:r0 + 64, hh * 512:hh * 512 + 512])

                # interleave next s-chunk's projections after first block
                if bl == 0 and ci < NCHUNK - 1:
                    emit_proj_stage(ci + 1, xch_next)
                    if ci < NCHUNK - 2:
                        xch_next = emit_x_dma(ci + 2)

            # -------- partial output projection for this query chunk ------
            for qi in range(4):
                for nc2 in range(2):
                    ps = p_av.tile([128, 512], F32, tag="av",
                                   name=f"psy_{ci}_{qi}_{nc2}")
                    for dc in range(4):
                        nc.tensor.matmul(
                            ps[:],
                            OT[:, dc * S + ci * SC + qi * 128:
                               dc * S + ci * SC + (qi + 1) * 128],
                            wo[:, dc * 1024 + nc2 * 512:
                               dc * 1024 + (nc2 + 1) * 512],
                            start=(dc == 0), stop=(dc == 3))
                    yb = p_yb.tile([128, 512], F32, tag="yb")
                    nc.vector.tensor_add(
                        yb[:], ps[:], biasbc[:, nc2 * 512:(nc2 + 1) * 512])
                    nc.sync.dma_start(
                        y_d.ap()[ci * SC + qi * 128:ci * SC + (qi + 1) * 128,
                                 nc2 * 512:(nc2 + 1) * 512], yb[:])

    nc.compile()
    return nc


def _get_program():
    if 'nc' not in _CACHE:
        _CACHE['nc'] = _build_program()
    return _CACHE['nc']


def _tri_masks():
    # masks[r] = [128, 2 x 512]: per hh half, [zeros(128r) | tri | ones]
    import ml_dtypes
    p = np.arange(128)[:, None]
    f = np.arange(512)[None, :]
    out = np.zeros((128, 4 * 1024), np.float32)
    for r in range(4):
        m = (p <= f - 128 * r).astype(np.float32)  # valid: key<=query
        out[:, r * 1024:r * 1024 + 512] = m
        out[:, r * 1024 + 512:(r + 1) * 1024] = m
    return out.astype(ml_dtypes.bfloat16)


def kernel(x, w_q, w_k, w_v, w_o, b_o):
    import ml_dtypes
    from concourse.bass_utils import run_bass_kernel_spmd

    BF = ml_dtypes.bfloat16
    x = np.asarray(x, dtype=np.float32)
    nc = _get_program()

    scale = np.float32(1.0 / np.sqrt(DK))
    wqT = np.ascontiguousarray(np.asarray(w_q, np.float32).T * scale)
    wkT = np.ascontiguousarray(np.asarray(w_k, np.float32).T)
    wvT = np.ascontiguousarray(np.asarray(w_v, np.float32).T)
    woT = np.ascontiguousarray(np.asarray(w_o, np.float32).T)
    bias_half = (np.asarray(b_o, np.float32) * 0.5)[None, :]
    masks = _tri_masks()

    xTs = [np.ascontiguousarray(x[b].T).astype(BF) for b in range(B)]
    in_maps = []
    for c in range(NCORES):
        b, p = c // 2, c % 2
        sl = slice(p * 512, (p + 1) * 512)
        in_maps.append({
            "xT": xTs[b],
            "wqT": wqT[:, sl].astype(BF),
            "wkT": wkT[:, sl].astype(BF),
            "wvT": wvT[:, sl].astype(BF),
            "woT": np.ascontiguousarray(woT[sl, :]).astype(BF),
            "bias": bias_half.astype(BF),
            "masks": masks,
        })

    res = run_bass_kernel_spmd(nc, in_maps, core_ids=list(range(NCORES)),
                               trace=_CACHE.get('trace', False),
                               tmpdir=_CACHE.get('tmpdir'))
    _CACHE['last_res'] = res

    y = np.empty((B, S, D), dtype=np.float32)
    for b in range(B):
        y[b] = res.results[2 * b]["y"] + res.results[2 * b + 1]["y"]
    return y
